# revision 9
# baseline (speedup 1.0000x reference)
# kernel.py — self-contained Trainium2 Bass kernel for nn_BTDG_31774168055963 (moe_routing)
#
# Reference computation (see problem):
#   branch1: x1 = BN(S1[s]); pe1 = einsum('be,bef->bf', x1, (P1[p] @ G1.reshape(rd,ed*ed)).reshape(-1,ed,ed))
#            pe1 = BN(pe1); pred1 = pe1 @ O1.T
#   branch2: x2 = BN(S2[s]); m1 = x2*T_S[times]; per-coarse-bucket Tucker core G2[c]
#            pe2 = sum_c [c==fine2coarse[times]] einsum(m1, (P2[p] @ G2[c].reshape(rd,ed*ed)).reshape(-1,ed,ed))
#            pe2 = BN(pe2 * T_O[times]); pred2 = pe2 @ O2.T
#   out = sigmoid(pred1 + pred2)
#
# Strategy (8 NeuronCores), v3:
#   - shard the Tucker rank dim rd=200 -> 25 per core; host sorts samples by
#     coarse bucket; branch-2 does per-bucket piece matmuls (<=512 cols).
#   - PE p-state warm-up matmuls at t=0 (clock ramps 0.65->2.4GHz over ~3us).
#   - G2 stream split across all 3 DMA rings (sync/scalar/gpsimd), prefetch
#     depth 5, so branch 2 is never DMA-starved.
#   - ALL pb row broadcasts via gpsimd partition_broadcast: pb rows staged on
#     partitions {0,32,64,96}, DVE-copied row->partition 0, broadcast (no DMA
#     broadcasts): branch 1 is fully DMA-free and both AllReduces run in
#     DMA-quiet windows (collectives defer under active DMA).
#   - branch-1 needs (G1, x1, pb1 stage) prefetched during branch 2; O-chunks
#     + pe2 readback DMA during branch 1, gated behind AR-pe2.
#   - order: branch2 -> evict+AR(pe2) hidden under branch1 -> evict+AR(pe1)
#     hidden under logits PASS A (pe2-only partial logits staged to SBUF bf16),
#     then PASS B re-injects partials into PSUM and adds the pe1 contribution,
#     sigmoid chunk-pipelined, bf16 output store, host concat+unpermute.
#   - all matmuls bf16 (fp32 PSUM accumulation); BN statistics fp32.

import numpy as np
import ml_dtypes

BF16 = ml_dtypes.bfloat16

B, E, R2, T, C, ED, RD = 2048, 20000, 500, 365, 12, 200, 200
NCORES = 8
RS = RD // NCORES       # 25 r's per core
ES = E // NCORES        # 2500 vocab per core
BN_EPS = 1e-5
NSLOT = 7               # ceil(RS/4) pb-stage slots

_cache = {}


def _build(pieces, debug=False):
    """Build + compile the per-core bass kernel. `pieces` is a tuple of
    (coarse_id, col_off, col_len) for branch-2 bucket matmuls (<=512 cols each)."""
    import concourse.bass as bass
    import concourse.mybir as mybir
    import concourse.tile as tile
    from concourse import bacc

    f32 = mybir.dt.float32
    bf16 = mybir.dt.bfloat16

    nc = bacc.Bacc("TRN2", target_bir_lowering=False, debug=False, num_devices=NCORES)

    # ---------------- I/O ----------------
    x1_in = nc.dram_tensor("x1_in", [ED, B], bf16, kind="ExternalInput")   # S1[s_p].T
    x2_in = nc.dram_tensor("x2_in", [ED, B], bf16, kind="ExternalInput")   # S2[s_p].T
    ts_in = nc.dram_tensor("ts_in", [ED, B], bf16, kind="ExternalInput")   # T_S[times_p].T
    to_in = nc.dram_tensor("to_in", [ED, B], bf16, kind="ExternalInput")   # T_O[times_p].T
    g1_in = nc.dram_tensor("g1_in", [100, RS, 2, ED], bf16, kind="ExternalInput")
    g2_in = nc.dram_tensor("g2_in", [RS, 2, 100, C, ED], bf16, kind="ExternalInput")
    pb1_in = nc.dram_tensor("pb1_in", [RS, B], bf16, kind="ExternalInput")
    pb1s_in = nc.dram_tensor("pb1s_in", [4, 3, B], bf16, kind="ExternalInput")  # rows 16..24 staged
    # pb2 rows r=4q+j live at [j, q, :] (staged on partitions {0,32,64,96})
    pb2_in = nc.dram_tensor("pb2_in", [4, NSLOT, B], bf16, kind="ExternalInput")
    # O chunks: feat layout {0:128, 128:200} x {O1, O2}
    oc0_in = nc.dram_tensor("oc0_in", [128, ES], bf16, kind="ExternalInput")
    oc1_in = nc.dram_tensor("oc1_in", [72, ES], bf16, kind="ExternalInput")
    oc2_in = nc.dram_tensor("oc2_in", [100, ES], bf16, kind="ExternalInput")
    oc3_in = nc.dram_tensor("oc3_in", [100, ES], bf16, kind="ExternalInput")
    bnp_in = nc.dram_tensor("bnp_in", [ED, 8], f32, kind="ExternalInput")  # g11,b11,g12,b12,g21,b21,g22,b22
    out_t = nc.dram_tensor("out", [ES, B], bf16, kind="ExternalOutput")

    FS = [(0, 128), (128, 72)]    # branch-1 / pe1 feat M-tiles (offset, len)
    FS2 = [(0, 100), (100, 100)]  # branch-2 / pe2 feat M-tiles

    with tile.TileContext(nc) as tc:
        from contextlib import ExitStack
        with ExitStack() as ctx:
            singles = ctx.enter_context(tc.tile_pool(name="singles", bufs=1))
            small = ctx.enter_context(tc.tile_pool(name="small", bufs=4))
            btmp = ctx.enter_context(tc.tile_pool(name="btmp", bufs=2))
            perst = ctx.enter_context(tc.tile_pool(name="perst", bufs=1))
            dram = ctx.enter_context(tc.tile_pool(name="dram", bufs=1, space="DRAM"))

            # ---- scalar queue: warm the Sqrt activation table immediately ----
            dummy = singles.tile([1, 1], f32)
            nc.vector.memset(dummy, 1.0)
            dummy2 = singles.tile([1, 1], f32)
            nc.scalar.activation(dummy2[:], dummy[:],
                                 mybir.ActivationFunctionType.Sqrt, scale=1.0)

            bnp100 = singles.tile([100, 2, 8], f32)
            nc.scalar.dma_start(bnp100[:], bnp_in.rearrange("(h p) c -> p h c", p=100))
            bnpA = singles.tile([128, 8], f32)
            nc.scalar.dma_start(bnpA[:], bnp_in[0:128, :])
            bnpB = singles.tile([72, 8], f32)
            nc.scalar.dma_start(bnpB[:], bnp_in[128:200, :])
            eps100 = singles.tile([100, 1], f32)
            nc.vector.memset(eps100, BN_EPS)
            eps128 = singles.tile([128, 1], f32)
            nc.vector.memset(eps128, BN_EPS)

            def bn_normalize(src_ap, dst_tile, gcol, bcol, eps_tile,
                             postmul=None, premul=None):
                """dst = BN(src [* premul]) * g + b [* postmul] — batch stats along free dim."""
                P = dst_tile.shape[0]
                if premul is not None:
                    pre = btmp.tile([128, B], bf16, tag="bn_pre", bufs=1)
                    nc.vector.tensor_tensor(pre[:P], src_ap, premul, mybir.AluOpType.mult)
                    src_ap = pre[:P]
                stats = small.tile([128, 4, 6], f32, tag="bn_stats")
                for i in range(4):
                    nc.vector.bn_stats(stats[:P, i, :], src_ap[:, i * 512:(i + 1) * 512])
                mv = small.tile([128, 2], f32, tag="bn_mv")
                nc.vector.bn_aggr(mv[:P], stats[:P])
                rstd = small.tile([128, 1], f32, tag="bn_rstd")
                nc.scalar.activation(rstd[:P], mv[:P, 1:2], mybir.ActivationFunctionType.Sqrt,
                                     bias=eps_tile[:P], scale=1.0)
                nc.vector.reciprocal(rstd[:P], rstd[:P])
                A = small.tile([128, 1], f32, tag="bn_A")
                nc.vector.tensor_mul(A[:P], rstd[:P], gcol)
                Bt = small.tile([128, 1], f32, tag="bn_B")
                nc.vector.tensor_mul(Bt[:P], mv[:P, 0:1], A[:P])
                nc.vector.tensor_tensor(Bt[:P], bcol, Bt[:P], mybir.AluOpType.subtract)
                nc.vector.tensor_scalar(dst_tile[:], src_ap, A[:P], Bt[:P],
                                        mybir.AluOpType.mult, mybir.AluOpType.add)
                if postmul is not None:
                    nc.vector.tensor_tensor(dst_tile[:], dst_tile[:], postmul,
                                            mybir.AluOpType.mult)

            # DRAM bounce buffers for the two AllReduces
            pe1_dram = dram.tile([ED, B], bf16, name="pe1_src")
            pe1_out_dram = dram.tile([ED, B], bf16, name="pe1_dst", addr_space="Shared")
            pe2_dram = dram.tile([ED, B], bf16, name="pe2_src")
            pe2_out_dram = dram.tile([ED, B], bf16, name="pe2_dst", addr_space="Shared")

            pe_bn = [None, None, None, None]
            oc_sb = []

            # ================= Tucker phase (scoped pools) =================
            with tc.tile_pool(name="tk", bufs=1) as tk, \
                 tc.tile_pool(name="psum_tk", bufs=1, space="PSUM") as pst, \
                 tc.tile_pool(name="stg", bufs=1) as stg, \
                 tc.tile_pool(name="bcast", bufs=3) as bcpool, \
                 tc.tile_pool(name="zpool", bufs=3) as zpool, \
                 tc.tile_pool(name="gw", bufs=6) as gwpool:

                # ---- startup DMAs ----
                # scalar ring: pb2 stage rows first (branch-2 critical path)
                stage2 = stg.tile([128, NSLOT, B], bf16, tag="stage_big")
                for j in range(4):
                    nc.scalar.dma_start(stage2[32 * j:32 * j + 1, :, :], pb2_in[j:j + 1])
                # sync ring: x2/ts h0 first (m1t_0 unblocks the first matmul),
                # then g2 r0h0 ahead of the h1 inputs
                x2raw = []
                tsraw = []
                g2c_r0 = []
                for h in range(2):
                    r2t = tk.tile([100, B], bf16, tag="rawx", bufs=2)
                    nc.sync.dma_start(r2t[:], x2_in[100 * h:100 * (h + 1), :])
                    x2raw.append(r2t)
                    tst = tk.tile([100, B], bf16, name=f"tsraw_{h}")
                    nc.sync.dma_start(tst[:], ts_in[100 * h:100 * (h + 1), :])
                    tsraw.append(tst)
                    g2c = gwpool.tile([100, C, ED], bf16, tag="g2w")
                    nc.sync.dma_start(g2c[:], g2_in[0, h])
                    g2c_r0.append(g2c)

                ps2_a = pst.tile([100, B], f32, tag="ps_m0", name="ps2_a")
                ps2_b = pst.tile([100, B], f32, tag="ps_m1", name="ps2_b")
                ps2 = [ps2_a, ps2_b]

                # ---- PE p-state warm-up: matmuls on a zeroed tile into ps2
                # (results overwritten by the memset below) ----
                warm = tk.tile([128, 512], bf16)
                nc.vector.memset(warm, 0.0)
                for i in range(14):
                    nc.tensor.matmul(ps2_a[:, 0:512], lhsT=warm[:, 0:100],
                                     rhs=warm[:], start=True, stop=True)
                nc.vector.memset(ps2_a[:], 0.0)
                nc.vector.memset(ps2_b[:], 0.0)

                # ---- m1 = BN(x2) * ts  (branch-2 input, critical path) ----
                m1t = []
                for h in range(2):
                    d2 = tk.tile([100, B], bf16, name=f"m1t_{h}")
                    bn_normalize(x2raw[h][:], d2, bnp100[:, h, 4:5], bnp100[:, h, 5:6],
                                 eps100, postmul=tsraw[h][:])
                    m1t.append(d2)

                def bcast_row(stage, r_slot, r_part):
                    """stage row (partition 32*r_part, slot r_slot) -> [100,B] bcast tile."""
                    tmp = small.tile([1, B], bf16, tag="bc_tmp", bufs=2)
                    nc.vector.tensor_copy(tmp[:], stage[32 * r_part:32 * r_part + 1, r_slot, :])
                    pb = bcpool.tile([100, B], bf16, tag="ppb")
                    nc.gpsimd.partition_broadcast(pb[:], tmp[:])
                    return pb

                # ---------- branch 2: per-bucket pieces, G2 streamed on 3 rings ----------
                x1raw = []
                g1_sb = tk.tile([100, RS, 2, ED], bf16)
                to_sb = []
                for r in range(RS):
                    pb = bcast_row(stage2, r // 4, r % 4)
                    for h in range(2):
                        if r == 0:
                            g2c = g2c_r0[h]
                        else:
                            g2c = gwpool.tile([100, C, ED], bf16, tag="g2w")
                            if h == 0:
                                nc.sync.dma_start(g2c[:], g2_in[r, h])
                            elif r % 2 == 0:
                                nc.scalar.dma_start(g2c[:], g2_in[r, h])
                            else:
                                nc.gpsimd.dma_start(g2c[:], g2_in[r, h])
                        z = zpool.tile([100, B], bf16, tag="z")
                        nc.vector.tensor_tensor(z[:], m1t[h][:], pb[:], mybir.AluOpType.mult)
                        last = (r == RS - 1 and h == 1)
                        for mi, (mo, ml) in enumerate(FS2):
                            for (cid, off, ln) in pieces:
                                nc.tensor.matmul(
                                    ps2[mi][:, off:off + ln],
                                    lhsT=g2c[:, cid, mo:mo + ml],
                                    rhs=z[:, off:off + ln],
                                    start=False, stop=last,
                                    skip_group_check=True)
                    # prefetches for branch1, spread one-per-r over the scalar ring
                    if r in (2, 3):
                        t = tk.tile([100, B], bf16, tag="rawx", bufs=2)
                        nc.scalar.dma_start(t[:], x1_in[100 * (r - 2):100 * (r - 1), :])
                        x1raw.append(t)
                    elif 8 <= r < 13:
                        r5 = r - 8
                        nc.scalar.dma_start(
                            g1_sb[:, r5 * 5:(r5 + 1) * 5],
                            g1_in[:, r5 * 5:(r5 + 1) * 5])
                    elif r in (18, 19):
                        t = perst.tile([100, B], bf16, name=f"to_{r - 18}")
                        nc.scalar.dma_start(t[:], to_in[100 * (r - 18):100 * (r - 17), :])
                        to_sb.append(t)

                # x1 input BN (during late branch 2; DVE has slack)
                x1t = []
                for h in range(2):
                    d1 = tk.tile([100, B], bf16, name=f"x1t_{h}")
                    bn_normalize(x1raw[h][:], d1, bnp100[:, h, 0:1], bnp100[:, h, 1:2],
                                 eps100)
                    x1t.append(d1)

                # reload stage slots with pb1 rows 16..24 (tiny DMA; WAR on the
                # last pb2 broadcast orders it at the end of branch 2)
                stage1t = stg.tile([128, 3, B], bf16, tag="stage_big")
                for j in range(4):
                    nc.scalar.dma_start(stage1t[32 * j:32 * j + 1, :, :], pb1s_in[j:j + 1])

                # ---- evict branch 2 -> DRAM (bf16) and AllReduce (hidden under branch 1) ----
                for mi, (mo, ml) in enumerate(FS2):
                    pe_sb2 = btmp.tile([128, B], bf16, tag="pe_evict")
                    nc.vector.tensor_copy(pe_sb2[:ml], ps2[mi][:])
                    nc.sync.dma_start(pe2_dram[mo:mo + ml, :], pe_sb2[:ml])
                nc.gpsimd.collective_compute(
                    "AllReduce", mybir.AluOpType.add,
                    replica_groups=[list(range(NCORES))],
                    ins=[pe2_dram.opt()], outs=[pe2_out_dram.opt()])

                # pe2 readback + O-chunk loads (scalar queue: gated on AR-pe2,
                # run mid-branch-1 when the wire is free again)
                pe2_raw = []
                for mi, (mo, ml) in enumerate(FS2):
                    raw = perst.tile([100, B], bf16, name=f"pe2_raw{mi}")
                    nc.scalar.dma_start(raw[:], pe2_out_dram[mo:mo + ml, :])
                    pe2_raw.append(raw)
                for i, (oin, P) in enumerate([(oc0_in, 128), (oc1_in, 72),
                                              (oc2_in, 100), (oc3_in, 100)]):
                    t = perst.tile([P, ES], bf16, name=f"oc_{i}")
                    nc.scalar.dma_start(t[:], oin[:])
                    oc_sb.append(t)

                # ---------- branch 1 (fully DMA-free) ----------
                ps1_a = pst.tile([128, B], f32, tag="ps_m0", name="ps1_a")
                ps1_b = pst.tile([72, B], f32, tag="ps_m1", name="ps1_b")
                ps1 = [ps1_a, ps1_b]
                for r in range(RS):
                    if r < 16:
                        pb1 = bcpool.tile([100, B], bf16, tag="ppb")
                        nc.sync.dma_start(pb1[0:64], pb1_in[r:r + 1, :].partition_broadcast(64).squeeze(1))
                        nc.sync.dma_start(pb1[64:100], pb1_in[r:r + 1, :].partition_broadcast(36).squeeze(1))
                    else:
                        pb1 = bcast_row(stage1t, (r - 16) // 4, (r - 16) % 4)
                    for h in range(2):
                        z1 = zpool.tile([100, B], bf16, tag="z")
                        nc.vector.tensor_tensor(z1[:], x1t[h][:], pb1[:], mybir.AluOpType.mult)
                        first = (r == 0 and h == 0)
                        last = (r == RS - 1 and h == 1)
                        for mi, (mo, ml) in enumerate(FS):
                            for bc in range(4):
                                nc.tensor.matmul(
                                    ps1[mi][:, bc * 512:(bc + 1) * 512],
                                    lhsT=g1_sb[:, r, h, mo:mo + ml],
                                    rhs=z1[:, bc * 512:(bc + 1) * 512],
                                    start=first, stop=last)
                    if r == 20:
                        # BN(pe2 * T_O): DVE reaches this ~80%% into branch 1,
                        # after AR-pe2 + readback have landed — no DVE stall
                        for mi, (mo, ml) in enumerate(FS2):
                            dst = perst.tile([100, B], bf16, name=f"pebn_1_{mi}")
                            bn_normalize(pe2_raw[mi][:], dst, bnp100[:, mi, 6:7],
                                         bnp100[:, mi, 7:8], eps100, premul=to_sb[mi][:])
                            pe_bn[2 + mi] = (dst, 100)

                # ---- evict branch 1 + AllReduce (hidden under logits pass A) ----
                for mi, (mo, ml) in enumerate(FS):
                    pe_sb = btmp.tile([128, B], bf16, tag="pe_evict")
                    nc.vector.tensor_copy(pe_sb[:ml], ps1[mi][:])
                    nc.sync.dma_start(pe1_dram[mo:mo + ml, :], pe_sb[:ml])
                nc.gpsimd.collective_compute(
                    "AllReduce", mybir.AluOpType.add,
                    replica_groups=[list(range(NCORES))],
                    ins=[pe1_dram.opt()], outs=[pe1_out_dram.opt()])

            # ---------- logits: two passes over vocab tiles ----------
            # smallest tile first so the final tile's tail is minimal
            vts = [(0, ES - 19 * 128)] + [(ES - (19 - k) * 128, 128) for k in range(19)]
            with tc.tile_pool(name="partials", bufs=1) as ppool, \
                 tc.tile_pool(name="logits", bufs=3) as lpool, \
                 tc.tile_pool(name="psum_l", bufs=2, space="PSUM") as psl:

                partials = {}

                def passA(ti):
                    vo, vl = vts[ti]
                    psu = psl.tile([128, B], f32, tag="ps_l")
                    for bc in range(4):
                        for j, kc in enumerate((2, 3)):
                            peb, kl = pe_bn[kc]
                            nc.tensor.matmul(
                                psu[:vl, bc * 512:(bc + 1) * 512],
                                lhsT=oc_sb[kc][:kl, vo:vo + vl],
                                rhs=peb[:kl, bc * 512:(bc + 1) * 512],
                                start=(j == 0), stop=(j == 1))
                    part = ppool.tile([128, B], bf16, name=f"part_{ti}")
                    nc.vector.tensor_copy(part[:vl], psu[:vl])
                    partials[ti] = part

                def passB(ti):
                    vo, vl = vts[ti]
                    psu = psl.tile([128, B], f32, tag="ps_l")
                    nc.vector.tensor_copy(psu[:vl], partials[ti][:vl])
                    orow = lpool.tile([128, B], bf16, tag="orow", bufs=2)
                    for bc in range(4):
                        for j, kc in enumerate((0, 1)):
                            peb, kl = pe_bn[kc]
                            nc.tensor.matmul(
                                psu[:vl, bc * 512:(bc + 1) * 512],
                                lhsT=oc_sb[kc][:kl, vo:vo + vl],
                                rhs=peb[:kl, bc * 512:(bc + 1) * 512],
                                start=False, stop=(j == 1),
                                skip_group_check=True)
                        nc.scalar.activation(orow[:vl, bc * 512:(bc + 1) * 512],
                                             psu[:vl, bc * 512:(bc + 1) * 512],
                                             mybir.ActivationFunctionType.Sigmoid)
                        nc.sync.dma_start(out_t[vo:vo + vl, bc * 512:(bc + 1) * 512],
                                          orow[:vl, bc * 512:(bc + 1) * 512])

                # pass A head: AR-pe1 + readback + BN-pe1 land underneath
                pe1raws = []
                for ti in range(14):
                    passA(ti)
                    if ti == 2:
                        # pe1 readback + BN emitted early enough that the DVE
                        # reaches them around when AR-pe1 lands
                        for mi, (mo, ml) in enumerate(FS):
                            raw = lpool.tile([128, B], bf16, tag=f"pe1_raw{mi}", bufs=1)
                            nc.scalar.dma_start(raw[:ml], pe1_out_dram[mo:mo + ml, :])
                            pe1raws.append(raw)
                        nc.scalar.activation(dummy2[:], dummy[:],
                                             mybir.ActivationFunctionType.Sigmoid, scale=1.0)
                    if ti == 10:
                        for mi, (mo, ml) in enumerate(FS):
                            raw = pe1raws[mi]
                            par = bnpA if mi == 0 else bnpB
                            dst = perst.tile([128, B], bf16, name=f"pebn_0_{mi}")
                            bn_normalize(raw[:ml], dst[:ml], par[:, 2:3], par[:, 3:4],
                                         eps128)
                            pe_bn[mi] = (dst, ml)
                # interleave: spread pass-B output DMAs across the window
                for k in range(6):
                    passB(k)
                    passA(14 + k)
                for ti in range(6, 20):
                    passB(ti)

    nc.compile()
    return nc


def kernel(s, p, o, times, fine2coarse, S1, O1, S2, O2, P1, P2, G1, G2, T_S, T_O,
           g11, b11, g12, b12, g21, b21, g22, b22):
    from concourse.bass_utils import run_bass_kernel_spmd

    s = np.asarray(s); p = np.asarray(p); times = np.asarray(times)
    fine2coarse = np.asarray(fine2coarse)

    # ----- host-side routing (index logistics only) -----
    c = fine2coarse[times]                       # [B] coarse id per sample
    perm = np.argsort(c, kind="stable")
    c_sorted = c[perm]
    counts = np.bincount(c_sorted, minlength=C)
    offs = np.concatenate([[0], np.cumsum(counts)])
    pieces = []
    for cid in range(C):
        pos, en = int(offs[cid]), int(offs[cid + 1])
        while pos < en:
            nxt = min(en, pos + 512)     # moving-operand cap only; PSUM straddle is fine
            pieces.append((cid, pos, nxt - pos))
            pos = nxt
    pieces = tuple(pieces)

    key = pieces
    if key not in _cache:
        _cache[key] = _build(pieces)
    nc = _cache[key]

    s_p, p_p, t_p = s[perm], p[perm], times[perm]

    def bt(x):
        return np.ascontiguousarray(x, dtype=BF16)

    x1_in = bt(np.asarray(S1)[s_p].T)
    x2_in = bt(np.asarray(S2)[s_p].T)
    ts_in = bt(np.asarray(T_S)[t_p].T)
    to_in = bt(np.asarray(T_O)[t_p].T)
    pp1 = np.asarray(P1)[p_p]                       # [B, RD]
    pp2 = np.asarray(P2)[p_p]
    G1 = np.asarray(G1); G2 = np.asarray(G2)
    O1 = np.asarray(O1); O2 = np.asarray(O2)
    bnp = np.stack([g11, b11, g12, b12, g21, b21, g22, b22], axis=1).astype(np.float32)
    bnp = np.ascontiguousarray(bnp)

    def stagefmt(pp_rs):
        """[RS, B] rows -> [4, NSLOT, B] with row r=4q+j at [j, q]."""
        st = np.zeros((4, NSLOT, B), dtype=BF16)
        for r in range(RS):
            st[r % 4, r // 4] = pp_rs[r]
        return st

    in_maps = []
    for k in range(NCORES):
        rs = slice(RS * k, RS * (k + 1))
        vs = slice(ES * k, ES * (k + 1))
        g1k = bt(G1[rs].reshape(RS, 2, 100, ED).transpose(2, 0, 1, 3))
        g2k = bt(G2[:, rs].reshape(C, RS, 2, 100, ED).transpose(1, 2, 3, 0, 4))
        pb1 = bt(pp1[:, rs].T)
        pb1s = np.zeros((4, 3, B), dtype=BF16)
        for r in range(16, RS):
            pb1s[(r - 16) % 4, (r - 16) // 4] = pb1[r]
        pb2 = stagefmt(bt(pp2[:, rs].T))
        o1t = bt(O1[vs].T)   # [200, ES]
        o2t = bt(O2[vs].T)
        in_maps.append({
            "x1_in": x1_in, "x2_in": x2_in, "ts_in": ts_in, "to_in": to_in,
            "g1_in": g1k, "g2_in": g2k, "pb1_in": pb1, "pb1s_in": pb1s, "pb2_in": pb2,
            "oc0_in": np.ascontiguousarray(o1t[0:128]),
            "oc1_in": np.ascontiguousarray(o1t[128:200]),
            "oc2_in": np.ascontiguousarray(o2t[0:100]),
            "oc3_in": np.ascontiguousarray(o2t[100:200]),
            "bnp_in": bnp,
        })

    res = run_bass_kernel_spmd(nc, in_maps, core_ids=list(range(NCORES)))

    out_sorted = np.concatenate(
        [np.asarray(res.results[k]["out"], dtype=np.float32).T for k in range(NCORES)], axis=1)
    out = np.empty_like(out_sorted)
    out[perm] = out_sorted
    return out
